# revision 32
# baseline (speedup 1.0000x reference)
"""Trainium2 Bass kernel for a multi-head attention layer (B=4, S=1024, DIM=1024,
H=16 heads, DH=64) with RoPE on Q/K, unmasked softmax, and output projection.

Sharding: 8 cores = 4 batches x 2 query-halves. Each core computes K,V for all 16
heads of its batch (duplicated within each core pair - cheaper than any
collective at these sizes), Q for its 512 queries, attention, and the output
projection for its queries. No collectives. Host pre-transposes x / weights and
casts to bf16; the kernel emits the output transposed ([outdim, q]) and the host
transposes it back while assembling the full [B, S, DIM] output.

Layouts on device (per core):
  xT   [DIM, S]    x[b]^T            (bf16)
  xTq  [DIM, QS]   x[b]^T query cols (bf16)
  w*T  [DIM, DIM]  W^T (in-dim major) (bf16)
  csk/csq          cos/sin tables, 2-head-stacked [128, 2, S|QS] (bf16)
  r2T  [128, 128]  transposed block-diag rotate-half matrix (bf16)
  bcol [128, 3, 8] bq/bk/bo in [p, which, chunk] layout (f32, ACT bias operand)
  bv   [1, DIM]    (bf16, for the V bias matmul)
  outT [DIM, QS]   (bf16) final output transposed

All matmuls are out = lhsT.T @ rhs with the contraction dim on partitions.
Attention per head h (chunk mtq=h//2, partition offset poff=(h%2)*64):
  logits^T tiles: out[k-chunk 128, q 512] = kT_h_slice.T @ qT_h   (Kc=64; head
    pairs use disjoint PE row groups 0-63/64-127 and run concurrently)
  pT = exp(0.125 * logits^T)  (ScalarE, bf16 out)
  AV: out[65, q] = vA_h.T @ pT accumulated over k-chunks, where vA has a ones
    column appended -> row 64 = softmax denominator (V carries +bv so the
    normalized result includes the value bias exactly)
  normalize: copy av->SBUF f32 (frees PSUM), recip_approx of row 64, broadcast
    across partitions via a Kc=1 matmul, multiply. The recip/bcast/normalize
    chain of pair p is emitted after pair p+1's matmuls so it pipelines under
    PE work instead of stalling it.
Output projection accumulates over feature chunks; bo is folded into the
PSUM->SBUF copy on ScalarE (ACT bias). Result DMA'd out as [outdim, q].
"""

import os
import numpy as np
import ml_dtypes

import concourse.bass as bass
import concourse.mybir as mybir
import concourse.tile as tile
from concourse import bacc
from concourse.bass_utils import run_bass_kernel_spmd

B, S, DIM, H, DH = 4, 1024, 1024, 16, 64
QS = S // 2          # queries per core
P = 128
NCORES = 8
NCH = DIM // P       # 8 chunks of 128 along any DIM-sized axis
ROPE_THETA = 10000.0

BF16 = mybir.dt.bfloat16
F32 = mybir.dt.float32
AF = mybir.ActivationFunctionType
ALU = mybir.AluOpType

_CACHE = {}

LAST_EXEC_TIME_NS = None


def _maybe_install_trace_hook():
    """Install the NTFF profiling hook if tracing is requested (dev only)."""
    if not os.environ.get("BASS_TRACE"):
        return
    import sys, types
    if "antenv.axon_hooks" in sys.modules:
        return
    try:
        import antenv
        mod = types.ModuleType("antenv.axon_hooks")
        _state = {"hook": None}
        mod.set_axon_ntff_profile_hook = lambda h: _state.__setitem__("hook", h)
        mod.get_axon_ntff_profile_hook = lambda: _state["hook"]
        sys.modules["antenv.axon_hooks"] = mod
        antenv.axon_hooks = mod
        from trn_agent_boot.trn_boot import _ntff_profile_via_ctypes
        hook = _ntff_profile_via_ctypes("/opt/axon/libaxon_pjrt.so")
        if hook is not None:
            mod.set_axon_ntff_profile_hook(hook)
    except Exception:
        pass


def _build():
    nc = bacc.Bacc("TRN2", target_bir_lowering=False, debug=False,
                   num_devices=NCORES)

    xT = nc.dram_tensor("xT", [DIM, S], BF16, kind="ExternalInput").ap()
    wqT = nc.dram_tensor("wqT", [DIM, DIM], BF16, kind="ExternalInput").ap()
    wkT = nc.dram_tensor("wkT", [DIM, DIM], BF16, kind="ExternalInput").ap()
    wvT = nc.dram_tensor("wvT", [DIM, DIM], BF16, kind="ExternalInput").ap()
    woT = nc.dram_tensor("woT", [DIM, DIM], BF16, kind="ExternalInput").ap()
    csk = nc.dram_tensor("csk", [P, 2, S], BF16, kind="ExternalInput").ap()
    r2T = nc.dram_tensor("r2T", [P, P], BF16, kind="ExternalInput").ap()
    bcold = nc.dram_tensor("bcol", [P, 3, NCH], F32, kind="ExternalInput").ap()
    bvd = nc.dram_tensor("bv", [1, DIM], BF16, kind="ExternalInput").ap()
    outT = nc.dram_tensor("outT", [DIM, QS], BF16, kind="ExternalOutput").ap()

    with tile.TileContext(nc) as tc:
        with (
            tc.tile_pool(name="const", bufs=1) as constp,
            tc.tile_pool(name="persist", bufs=1) as pers,
            tc.tile_pool(name="f32t", bufs=6) as tmpp,
            tc.tile_pool(name="pT", bufs=4) as pTp,
            tc.tile_pool(name="avsb", bufs=4) as avsbp,
            tc.tile_pool(name="outc", bufs=3) as outp,
            tc.tile_pool(name="rcp", bufs=3) as rcpp,
            tc.tile_pool(name="psproj", bufs=2, space="PSUM") as psproj,
            tc.tile_pool(name="pslg", bufs=2, space="PSUM") as pslg,
            tc.tile_pool(name="pssm", bufs=2, space="PSUM") as pssm,
        ):
            # ---- constants ------------------------------------------------
            csk_sb = constp.tile([P, 2, S], BF16, tag="csk")
            nc.sync.dma_start(csk_sb[:], csk[:])
            r2T_sb = constp.tile([P, P], BF16, tag="r2T")
            nc.sync.dma_start(r2T_sb[:], r2T[:])
            bcol_sb = constp.tile([P, 3, NCH], F32, tag="bcol")
            nc.sync.dma_start(bcol_sb[:], bcold[:])
            bv_sb = constp.tile([1, DIM], BF16, tag="bv")
            nc.sync.dma_start(bv_sb[:], bvd[:])
            ones_bf = constp.tile([1, 512], BF16, tag="ones_bf")
            nc.vector.memset(ones_bf[:], 1.0)
            ones_f32 = constp.tile([1, DH], F32, tag="ones_f32")
            nc.vector.memset(ones_f32[:], 1.0)

            # ---- persistent activations / weights ------------------------
            xT_sb = pers.tile([P, NCH, S], BF16, tag="xT")
            wq_sb = pers.tile([P, NCH, DIM], BF16, tag="wq")
            wk_sb = pers.tile([P, NCH, DIM], BF16, tag="wk")
            wv_sb = pers.tile([P, NCH, DIM], BF16, tag="wv")
            wo_sb = pers.tile([P, NCH, DIM], BF16, tag="wo")
            kT_sb = pers.tile([P, NCH, S], BF16, tag="kT")
            qT_sb = pers.tile([P, NCH, QS], BF16, tag="qT")
            vA_sb = pers.tile([P, NCH, H, DH + 1], BF16, tag="vA")
            oT_sb = pers.tile([P, NCH, QS], BF16, tag="oT")

            # ones column of vA (the fused softmax denominator)
            nc.vector.memset(vA_sb[:, :, :, DH:DH + 1], 1.0)

            # chunked input DMAs, in the order compute consumes them
            for o in range(NCH):
                nc.sync.dma_start(xT_sb[:, o, :], xT[o * P:(o + 1) * P, :])
            for o in range(NCH):
                nc.sync.dma_start(wv_sb[:, o, :], wvT[o * P:(o + 1) * P, :])
            for o in range(NCH):
                nc.sync.dma_start(wk_sb[:, o, :], wkT[o * P:(o + 1) * P, :])
            for o in range(NCH):
                nc.sync.dma_start(wq_sb[:, o, :], wqT[o * P:(o + 1) * P, :])
            for o in range(NCH):
                nc.sync.dma_start(wo_sb[:, o, :], woT[o * P:(o + 1) * P, :])

            # ---- helper: projection + RoPE to a [dim-chunk, seq-slice] ----
            def proj_rope(out_sb, mt, ns, nw, w_sb, rhs_sb, bcol, cs_sb):
                """out_sb[:, mt, ns:ns+nw] = rope(W-chunk @ rhs + b)."""
                ps = psproj.tile([P, 512], F32, tag="proj", name="projps")
                acc = ps[:, :nw]
                for kc in range(NCH):
                    nc.tensor.matmul(
                        acc,
                        w_sb[:, kc, mt * P:(mt + 1) * P],
                        rhs_sb[:, kc, ns:ns + nw],
                        start=(kc == 0), stop=(kc == NCH - 1),
                    )
                # PSUM->SBUF with the bias folded in (ACT per-partition bias)
                zsb = tmpp.tile([P, 512], BF16, tag="f32t", name="zsb")[:, :nw]
                nc.scalar.activation(zsb, acc, AF.Identity,
                                     bias=bcol_sb[:, bcol, mt:mt + 1])
                rot = pssm.tile([P, 512], F32, tag="sm", name="rot")[:, :nw]
                nc.tensor.matmul(rot, r2T_sb[:], zsb, start=True, stop=True)
                t1 = tmpp.tile([P, 512], BF16, tag="f32t", name="t1")[:, :nw]
                nc.vector.tensor_mul(out=t1, in0=zsb,
                                     in1=cs_sb[:, 0, ns:ns + nw])
                t2 = tmpp.tile([P, 512], BF16, tag="f32t", name="t2")[:, :nw]
                nc.vector.tensor_mul(out=t2, in0=rot,
                                     in1=cs_sb[:, 1, ns:ns + nw])
                nc.vector.tensor_add(out=out_sb[:, mt, ns:ns + nw], in0=t1,
                                     in1=t2)

            # ---- V projection (+bv), packed into vA with ones column ------
            for mt in range(NCH):
                for nt in range(2):
                    ps = psproj.tile([P, 512], F32, tag="proj", name="vps")
                    acc = ps[:]
                    nc.tensor.matmul(acc, ones_bf[:, :P],
                                     bv_sb[:, nt * 512:(nt + 1) * 512],
                                     start=True, stop=False)
                    for kc in range(NCH):
                        nc.tensor.matmul(
                            acc,
                            xT_sb[:, kc, mt * P:(mt + 1) * P],
                            wv_sb[:, kc, nt * 512:(nt + 1) * 512],
                            start=False, stop=(kc == NCH - 1),
                        )
                    nc.vector.tensor_copy(
                        out=vA_sb[:, mt, nt * 8:(nt + 1) * 8, 0:DH],
                        in_=acc.rearrange("p (h d) -> p h d", h=8),
                    )

            # ---- attention, head pairs, finalize deferred by one pair -----
            NKG = 2   # k-chunks per logits psum tile / exp call

            def attn_core(hp):
                """logits + exp + AV + av->SBUF copy for head pair hp."""
                mtq = hp
                pts, avsbs = [], []
                for hip in range(2):
                    pt = pTp.tile([P, NCH, QS], BF16, tag="pT", name="pt")
                    pts.append(pt)
                for g in range(NCH // NKG):
                    lgs = []
                    for hip in range(2):
                        poff = hip * DH
                        lg = pslg.tile([P, 512 * NKG], F32, tag="lg",
                                       name="lg")
                        lgs.append(lg)
                        for j in range(NKG):
                            kt = g * NKG + j
                            nc.tensor.matmul(
                                lg[:, j * 512:(j + 1) * 512],
                                kT_sb[poff:poff + DH, mtq, kt * P:(kt + 1) * P],
                                qT_sb[poff:poff + DH, mtq, :],
                                start=True, stop=True,
                            )
                    for hip in range(2):
                        nc.scalar.activation(
                            pts[hip][:, g * NKG:(g + 1) * NKG, :],
                            lgs[hip].rearrange("p (j q) -> p j q", j=NKG),
                            AF.Exp, scale=0.125,
                        )
                for hip in range(2):
                    h = 2 * hp + hip
                    av = pssm.tile([P, 512], F32, tag="sm",
                                   name="av")[:DH + 1, :]
                    for kt in range(NCH):
                        nc.tensor.matmul(
                            av, vA_sb[:, kt, h, :], pts[hip][:, kt, :],
                            start=(kt == 0), stop=(kt == NCH - 1),
                        )
                    avsb = avsbp.tile([DH, QS], F32, tag="avsb",
                                      name="avsb")
                    nc.vector.tensor_copy(out=avsb[:], in_=av[0:DH, :])
                    den0 = rcpp.tile([1, QS], F32, tag="den0", name="den0")
                    nc.vector.tensor_copy(out=den0[:], in_=av[DH:DH + 1, :])
                    avsbs.append((avsb, den0))
                return avsbs

            def attn_finalize(hp, avsbs):
                """recip + partition-broadcast + normalize for head pair hp."""
                mtq = hp
                for hip in range(2):
                    poff = hip * DH
                    avsb, den0 = avsbs[hip]
                    rc = rcpp.tile([1, QS], F32, tag="rcp", name="rc")
                    nc.vector.reciprocal_approx_fast(out=rc[:], in_=den0[:])
                    bc = pssm.tile([P, 512], F32, tag="sm",
                                   name="bc")[:DH, :]
                    nc.tensor.matmul(bc, ones_f32[:], rc[:], start=True,
                                     stop=True)
                    nc.vector.tensor_mul(
                        out=oT_sb[poff:poff + DH, mtq, :],
                        in0=avsb[:], in1=bc,
                    )

            # K/Q projection of chunk hp interleaves with attention of pair
            # hp-1: PE projection matmuls fill the exp (ScalarE) latency.
            core_q = []   # (hp, avsbs) awaiting finalize
            for hp in range(H // 2):
                proj_rope(kT_sb, hp, 0, 512, wk_sb, xT_sb, 1, csk_sb)
                proj_rope(kT_sb, hp, 512, 512, wk_sb, xT_sb, 1, csk_sb)
                proj_rope(qT_sb, hp, 0, QS, wq_sb, xT_sb, 0, csk_sb)
                if hp > 0:
                    avsbs = attn_core(hp - 1)
                    core_q.append((hp - 1, avsbs))
                if len(core_q) > 1:
                    attn_finalize(*core_q.pop(0))
            avsbs = attn_core(H // 2 - 1)
            core_q.append((H // 2 - 1, avsbs))
            while core_q:
                attn_finalize(*core_q.pop(0))

            # ---- output projection (+bo via ACT bias) ---------------------
            for mt in range(NCH):
                ps = psproj.tile([P, 512], F32, tag="proj", name="ops")
                acc = ps[:]
                for fc in range(NCH):
                    nc.tensor.matmul(
                        acc, wo_sb[:, fc, mt * P:(mt + 1) * P],
                        oT_sb[:, fc, :], start=(fc == 0), stop=(fc == NCH - 1),
                    )
                osb = outp.tile([P, QS], BF16, tag="outc", name="osb")
                nc.scalar.activation(osb[:], acc, AF.Identity,
                                     bias=bcol_sb[:, 2, mt:mt + 1])
                nc.sync.dma_start(outT[mt * P:(mt + 1) * P, :], osb[:])

    nc.compile()
    return nc


def _host_tables():
    half = DH // 2
    freqs = 1.0 / (ROPE_THETA ** (np.arange(0, DH, 2, dtype=np.float64)[:half]
                                  / DH))
    ang = np.outer(np.arange(S, dtype=np.float64), freqs)      # (S, 32)
    cos64 = np.tile(np.cos(ang), (1, 2)).T.astype(np.float32)  # (64, S)
    sin64 = np.tile(np.sin(ang), (1, 2)).T.astype(np.float32)
    cos128 = np.concatenate([cos64, cos64], 0)
    sin128 = np.concatenate([sin64, sin64], 0)
    csk = np.ascontiguousarray(np.stack([cos128, sin128], 1))  # (128, 2, S)

    R64 = np.zeros((DH, DH), np.float32)
    for d in range(half):
        R64[d, d + half] = -1.0
        R64[d + half, d] = 1.0
    R2 = np.zeros((P, P), np.float32)
    R2[:DH, :DH] = R64
    R2[DH:, DH:] = R64
    return csk, np.ascontiguousarray(R2.T)


def kernel(x, Wq, bq, Wk, bk, Wv, bv, Wo, bo):
    global LAST_EXEC_TIME_NS
    _maybe_install_trace_hook()
    bf = ml_dtypes.bfloat16

    if "nc" not in _CACHE:
        _CACHE["nc"] = _build()
        _CACHE["tables"] = _host_tables()
    nc = _CACHE["nc"]
    csk, r2T = _CACHE["tables"]
    csk = csk.astype(bf)
    r2T = r2T.astype(bf)

    x = np.asarray(x, np.float32)
    xT = np.ascontiguousarray(x.transpose(0, 2, 1)).astype(bf)   # [B, DIM, S]
    wqT = np.ascontiguousarray(np.asarray(Wq, np.float32).T).astype(bf)
    wkT = np.ascontiguousarray(np.asarray(Wk, np.float32).T).astype(bf)
    wvT = np.ascontiguousarray(np.asarray(Wv, np.float32).T).astype(bf)
    woT = np.ascontiguousarray(np.asarray(Wo, np.float32).T).astype(bf)
    bcol = np.ascontiguousarray(
        np.stack([np.asarray(b, np.float32).reshape(NCH, P).T
                  for b in (bq, bk, bo)], 1))                    # [128, 3, 8]
    bvh = np.asarray(bv, np.float32).astype(bf).reshape(1, DIM)

    # Keys/values may be presented in any order (softmax and AV are
    # permutation-invariant over keys, and RoPE rides along via the equally
    # rolled cos/sin table), so roll each core's columns to put its queries
    # at 0:QS and drop the separate query-slice inputs.
    in_maps = []
    for c in range(NCORES):
        b, qh = c // 2, c % 2
        qoff = qh * QS
        in_maps.append({
            "xT": np.ascontiguousarray(np.roll(xT[b], -qoff, axis=1)),
            "wqT": wqT, "wkT": wkT, "wvT": wvT, "woT": woT,
            "csk": np.ascontiguousarray(np.roll(csk, -qoff, axis=2)),
            "r2T": r2T,
            "bcol": bcol, "bv": bvh,
        })

    res = run_bass_kernel_spmd(nc, in_maps, list(range(NCORES)))
    LAST_EXEC_TIME_NS = res.exec_time_ns

    out = np.empty((B, S, DIM), np.float32)
    for c in range(NCORES):
        b, qh = c // 2, c % 2
        out[b, qh * QS:(qh + 1) * QS, :] = (
            res.results[c]["outT"].astype(np.float32).T)
    return out
